# revision 9
# baseline (speedup 1.0000x reference)
"""ABCNN-1 block on 8 Trainium2 NeuronCores, data-parallel over batch.

Per batch (S=256, D=512, W=3):
  F0 = F0r * m0[:, None]; F1 = F1r * m1[:, None]          (masked, bf16)
  d2[i,j] = relu(||F0_i||^2 + ||F1_j||^2 - 2 F0_i.F1_j)   (PE, rank-1 augmented)
  A = 1/(1 + sqrt(d2))                                    (DVE/ACT)
  F0a = A^T W0 ; F1a = A W1                               (PE, bf16)
  out_c = avgpool3(tanh(conv3x1([Fc, Fca]) + cb))         (PE banded matmuls + ACT tanh)

Sharding: batch 64 -> 8 cores x 8 batches. Params replicated. No collectives.
"""

import sys

import numpy as np
import ml_dtypes

BF16 = ml_dtypes.bfloat16
B, S, D, W = 64, 256, 512, 3
H = S + W - 1  # 258 conv output length
NCORES = 8
BPC = B // NCORES
P = 128

_CACHE = {}


def _ensure_path():
    for p in ("/opt/trn_rl_repo",):
        if p not in sys.path:
            sys.path.insert(0, p)


def _build_graph():
    """Build the single-core Bass/Tile graph (SPMD: same program, 8 data shards)."""
    import os
    DBG = set(os.environ.get("KDBG", "").split(","))
    _ensure_path()
    import concourse.bacc as bacc
    import concourse.mybir as mybir
    from concourse import tile
    from concourse.tile_rust import add_dep_helper

    f32 = mybir.dt.float32
    bf16 = mybir.dt.bfloat16
    AF = mybir.ActivationFunctionType
    ALU = mybir.AluOpType

    nc = bacc.Bacc(None, target_bir_lowering=False)

    # ---------------- DRAM parameters ----------------
    F0r = nc.dram_tensor("F0r", [BPC, S, D], f32, kind="ExternalInput")
    F1r = nc.dram_tensor("F1r", [BPC, S, D], f32, kind="ExternalInput")
    m0 = nc.dram_tensor("m0", [BPC, S], f32, kind="ExternalInput")
    m1 = nc.dram_tensor("m1", [BPC, S], f32, kind="ExternalInput")
    w0 = nc.dram_tensor("w0bf", [S, D], bf16, kind="ExternalInput")
    w1 = nc.dram_tensor("w1bf", [S, D], bf16, kind="ExternalInput")
    c0t = nc.dram_tensor("c0t", [S, H], bf16, kind="ExternalInput")  # conv lhsT ch0
    c1t = nc.dram_tensor("c1t", [S, H], bf16, kind="ExternalInput")  # conv lhsT ch1
    ptd = nc.dram_tensor("ptd", [H, S], bf16, kind="ExternalInput")  # pool lhsT (1/3 band)
    idp1 = nc.dram_tensor("idp1", [P, P], bf16, kind="ExternalInput")  # +1 * I
    idf = nc.dram_tensor("idf", [P, P], f32, kind="ExternalInput")  # I fp32
    cbd = nc.dram_tensor("cb", [P, 1], f32, kind="ExternalInput")  # conv bias bcast
    onesd = nc.dram_tensor("ones1", [1, 2 * P], bf16, kind="ExternalInput")
    out0 = nc.dram_tensor("out0", [BPC, S, D], f32, kind="ExternalOutput")
    out1 = nc.dram_tensor("out1", [BPC, S, D], f32, kind="ExternalOutput")

    sqrt_instrs = []
    tanh_instrs = []

    with tile.TileContext(nc) as tc:
        with (
            tc.tile_pool(name="const", bufs=1) as cp,
            tc.tile_pool(name="keepF", bufs=2 * BPC + 1) as keepF,
            tc.tile_pool(name="keepA", bufs=BPC + 1) as keepA,
            tc.tile_pool(name="keepAT", bufs=BPC + 1) as keepAT,
        ):
            # ---------------- constants to SBUF ----------------
            W0s = cp.tile([P, 2 * D], bf16, tag="w0")
            W1s = cp.tile([P, 2 * D], bf16, tag="w1")
            C0s = cp.tile([P, 2 * H], bf16, tag="c0")
            C1s = cp.tile([P, 2 * H], bf16, tag="c1")
            PTs = cp.tile([P, 2 * S], bf16, tag="pt")
            PTt = cp.tile([2, S], bf16, tag="ptt")
            IDp1 = cp.tile([P, P], bf16, tag="idp1")
            IDf = cp.tile([P, P], f32, tag="idf")
            CBs = cp.tile([P, 1], f32, tag="cb")
            ONEr = cp.tile([1, 2 * P], bf16, tag="ones")
            for t in (0, 1):
                nc.sync.dma_start(W0s[:, t * D:(t + 1) * D], w0[t * P:(t + 1) * P, :])
                nc.sync.dma_start(W1s[:, t * D:(t + 1) * D], w1[t * P:(t + 1) * P, :])
                nc.sync.dma_start(C0s[:, t * H:(t + 1) * H], c0t[t * P:(t + 1) * P, :])
                nc.sync.dma_start(C1s[:, t * H:(t + 1) * H], c1t[t * P:(t + 1) * P, :])
                nc.sync.dma_start(PTs[:, t * S:(t + 1) * S], ptd[t * P:(t + 1) * P, :])
            nc.sync.dma_start(PTt[:], ptd[2 * P:2 * P + 2, :])
            nc.sync.dma_start(IDp1[:], idp1[:])
            nc.sync.dma_start(IDf[:], idf[:])
            nc.sync.dma_start(CBs[:], cbd[:])
            nc.sync.dma_start(ONEr[:], onesd[:])

            F0b_l, F1b_l, Ab_l, At_l = [], [], [], []

            # ================= PHASE 1: masks, transposes, d2, A =================
            with (
                tc.tile_pool(name="raw", bufs=4) as rawp,
                tc.tile_pool(name="mc", bufs=4) as mcp,
                tc.tile_pool(name="scr", bufs=3) as scrp,
                tc.tile_pool(name="sqc", bufs=4) as sqcp,
                tc.tile_pool(name="aug", bufs=4) as augp,
                tc.tile_pool(name="ft", bufs=4) as ftp,
                tc.tile_pool(name="aw", bufs=8) as awp,
                tc.tile_pool(name="tps", bufs=3, space="PSUM") as tpsp,
                tc.tile_pool(name="d2p", bufs=2, space="PSUM") as d2p,
            ):
                for b in range(BPC):
                    F0w = rawp.tile([P, 2 * D], f32, tag="raw")
                    F1w = rawp.tile([P, 2 * D], f32, tag="raw")
                    nc.sync.dma_start(F0w[:].rearrange("p (t d) -> p t d", t=2), F0r[b].rearrange("(t p) d -> p t d", p=P))
                    nc.sync.dma_start(F1w[:].rearrange("p (t d) -> p t d", t=2), F1r[b].rearrange("(t p) d -> p t d", p=P))
                    mc0 = mcp.tile([P, 2], f32, tag="mc")
                    mc1 = mcp.tile([P, 2], f32, tag="mc")
                    nc.sync.dma_start(mc0[:], m0[b].rearrange("(t p) -> p t", p=P))
                    nc.sync.dma_start(mc1[:], m1[b].rearrange("(t p) -> p t", p=P))

                    # masked, bf16 (DVE)
                    F0b = keepF.tile([P, 2 * D], bf16, tag="fk")
                    F1b = keepF.tile([P, 2 * D], bf16, tag="fk")
                    for t in (0, 1):
                        sl = slice(t * D, (t + 1) * D)
                        nc.vector.tensor_scalar_mul(F0b[:, sl], F0w[:, sl], mc0[:, t:t + 1])
                        nc.vector.tensor_scalar_mul(F1b[:, sl], F1w[:, sl], mc1[:, t:t + 1])
                    F0b_l.append(F0b)
                    F1b_l.append(F1b)

                    # row norms (DVE fused square+reduce)
                    sq0 = sqcp.tile([P, 2], f32, tag="sqc")
                    sq1 = sqcp.tile([P, 2], f32, tag="sqc")
                    for t in (0, 1):
                        sl = slice(t * D, (t + 1) * D)
                        scr = scrp.tile([P, D], bf16, tag="scr")
                        nc.scalar.activation(scr[:], F0b[:, sl], AF.Square,
                                             accum_out=sq0[:, t:t + 1])
                        scr2 = scrp.tile([P, D], bf16, tag="scr")
                        nc.scalar.activation(scr2[:], F1b[:, sl], AF.Square,
                                             accum_out=sq1[:, t:t + 1])

                    # transposes: F0T scaled by -2, F1T plain  (PE + ACT copies)
                    F0t = ftp.tile([P, 4 * S], bf16, tag="ft")
                    F1t = ftp.tile([P, 4 * S], bf16, tag="ft")
                    for src, dst, cscale in ((F0b, F0t, -2.0), (F1b, F1t, 1.0)):
                        for half in (0, 1):  # two d-chunk pairs
                            ps = tpsp.tile([P, 512], bf16, tag="tps")
                            for ci in (0, 1):  # chunk within pair
                                c = half * 2 + ci
                                for t in (0, 1):  # s-half
                                    nc.tensor.transpose(
                                        ps[:, ci * S + t * P: ci * S + (t + 1) * P],
                                        src[:, t * D + c * P: t * D + (c + 1) * P],
                                        IDp1[:])
                            nc.scalar.mul(dst[:, half * 512:(half + 1) * 512], ps[:], cscale)

                    # sq rows -> augmented rank-1 tiles
                    sq0r = augp.tile([1, 2 * P], bf16, tag="aug")
                    sq1r = augp.tile([1, 2 * P], bf16, tag="aug")
                    if "nosqt" in DBG:
                        pass
                    else:
                        sqr = tpsp.tile([1, 2 * P], f32, tag="sqr")
                        sqr1 = tpsp.tile([1, 2 * P], f32, tag="sqr")
                        for t in (0, 1):
                            nc.tensor.transpose(sqr[0:1, t * P:(t + 1) * P], sq0[:, t:t + 1], IDf[:])
                            nc.tensor.transpose(sqr1[0:1, t * P:(t + 1) * P], sq1[:, t:t + 1], IDf[:])
                        nc.vector.tensor_copy(sq0r[:], sqr[0:1, :])
                        nc.vector.tensor_copy(sq1r[:], sqr1[0:1, :])

                    # d2 = -2*F0.F1 + sq0_i + sq1_j   (PE, psum [128, 512])
                    d2 = d2p.tile([P, 2 * S], f32, tag="d2")
                    for it in (0, 1):
                        osl = slice(it * S, (it + 1) * S)
                        for c in range(4):
                            nc.tensor.matmul(
                                d2[:, osl],
                                F0t[:, c * S + it * P: c * S + (it + 1) * P],
                                F1t[:, c * S:(c + 1) * S],
                                start=(c == 0), stop=False)
                        if "nok1" in DBG:
                            nc.tensor.matmul(
                                d2[:, osl],
                                F0t[:, 3 * S + it * P: 3 * S + (it + 1) * P],
                                F1t[:, 3 * S:4 * S], start=False, stop=True)
                        else:
                            nc.tensor.matmul(
                                d2[:, osl], sq0r[0:1, it * P:(it + 1) * P], ONEr[:],
                                start=False, stop=False)
                            nc.tensor.matmul(
                                d2[:, osl], ONEr[0:1, it * P:(it + 1) * P], sq1r[:],
                                start=False, stop=True)

                    # A = 1/(1+sqrt(relu(d2)))
                    Af = awp.tile([P, 2 * S], f32, tag="aw")
                    nc.vector.tensor_scalar_max(Af[:], d2[:], 0.0)
                    An = awp.tile([P, 2 * S], f32, tag="aw")
                    si = nc.scalar.activation(An[:], Af[:], AF.Sqrt)
                    sqrt_instrs.append(si)
                    An1 = awp.tile([P, 2 * S], f32, tag="aw")
                    nc.vector.tensor_scalar_add(An1[:], An[:], 1.0)
                    Ar = awp.tile([P, 2 * S], f32, tag="aw")
                    if "norecip" in DBG:
                        nc.vector.reciprocal(Ar[:], An1[:])
                    else:
                        nc.vector.reciprocal_approx_fast(out=Ar[:], in_=An1[:])
                    Ab = keepA.tile([P, 2 * S], bf16, tag="ak")
                    nc.scalar.copy(Ab[:], Ar[:])
                    Ab_l.append(Ab)

                    # AT (PE transpose of A blocks)
                    At = keepAT.tile([P, 2 * S], bf16, tag="atk")
                    psA = tpsp.tile([P, 512], bf16, tag="tps")
                    for jt in (0, 1):
                        for it in (0, 1):
                            nc.tensor.transpose(
                                psA[:, jt * S + it * P: jt * S + (it + 1) * P],
                                Ab[:, it * S + jt * P: it * S + (jt + 1) * P],
                                IDp1[:])
                    nc.scalar.copy(At[:], psA[:])
                    At_l.append(At)

            # ================= PHASE 2: F0a/F1a, conv, tanh, pool =================
            with (
                tc.tile_pool(name="fas", bufs=4) as fasp,
                tc.tile_pool(name="Tp", bufs=4) as Tp,
                tc.tile_pool(name="Tt", bufs=3) as Ttp,
                tc.tile_pool(name="osb", bufs=4) as osp,
                tc.tile_pool(name="fap", bufs=2, space="PSUM") as fap,
                tc.tile_pool(name="Gp", bufs=2, space="PSUM") as gp,
            ):
                for b in range(BPC):
                    F0b, F1b, Ab, At = F0b_l[b], F1b_l[b], Ab_l[b], At_l[b]

                    # F0a = A^T W0 -> [j, d]; F1a = A W1 -> [i, d]
                    fa0 = fap.tile([P, 2 * D], f32, tag="fa")
                    fa1 = fap.tile([P, 2 * D], f32, tag="fa")
                    for mt in (0, 1):
                        osl = slice(mt * D, (mt + 1) * D)
                        for kt in (0, 1):
                            nc.tensor.matmul(
                                fa0[:, osl],
                                Ab[:, kt * S + mt * P: kt * S + (mt + 1) * P],
                                W0s[:, kt * D:(kt + 1) * D],
                                start=(kt == 0), stop=(kt == 1))
                            nc.tensor.matmul(
                                fa1[:, osl],
                                At[:, kt * S + mt * P: kt * S + (mt + 1) * P],
                                W1s[:, kt * D:(kt + 1) * D],
                                start=(kt == 0), stop=(kt == 1))
                    F0a = fasp.tile([P, 2 * D], bf16, tag="fas")
                    F1a = fasp.tile([P, 2 * D], bf16, tag="fas")
                    nc.scalar.copy(F0a[:], fa0[:])
                    nc.vector.tensor_copy(F1a[:], fa1[:])

                    # conv (banded matmuls) + tails, then tanh
                    Gt = fap.tile([2, 2 * D], f32, tag="fa")
                    T_l = []
                    for ci, (Fb, Fa, Cs) in enumerate(((F0b, F0a, None), (F1b, F1a, None))):
                        G = gp.tile([P, 2 * D], f32, tag="G")
                        # h-tile0 (h 0..127): only s-tile0
                        nc.tensor.matmul(G[:, 0:D], C0s[:, 0:P], Fb[:, 0:D],
                                         start=True, stop=False)
                        nc.tensor.matmul(G[:, 0:D], C1s[:, 0:P], Fa[:, 0:D],
                                         start=False, stop=True)
                        # h-tile1 (h 128..255): s-tile0 (rows 126/127) + s-tile1
                        nc.tensor.matmul(G[:, D:2 * D], C0s[:, P:S], Fb[:, 0:D],
                                         start=True, stop=False)
                        nc.tensor.matmul(G[:, D:2 * D], C0s[:, H + P:H + S], Fb[:, D:2 * D],
                                         start=False, stop=False)
                        nc.tensor.matmul(G[:, D:2 * D], C1s[:, P:S], Fa[:, 0:D],
                                         start=False, stop=False)
                        nc.tensor.matmul(G[:, D:2 * D], C1s[:, H + P:H + S], Fa[:, D:2 * D],
                                         start=False, stop=True)
                        # tail (h 256..257): s-tile1 only, into shared [2, 1024] psum
                        tsl = slice(ci * D, (ci + 1) * D)
                        nc.tensor.matmul(Gt[0:2, tsl], C0s[:, H + S:2 * H], Fb[:, D:2 * D],
                                         start=True, stop=False)
                        nc.tensor.matmul(Gt[0:2, tsl], C1s[:, H + S:2 * H], Fa[:, D:2 * D],
                                         start=False, stop=True)
                        T = Tp.tile([P, 2 * D], bf16, tag="T")
                        ti = nc.scalar.activation(T[:], G[:], AF.Tanh, bias=CBs[:, 0:1])
                        tanh_instrs.append(ti)
                        T_l.append(T)
                    Tt = Ttp.tile([2, 2 * D], bf16, tag="tt")
                    ti = nc.scalar.activation(Tt[:], Gt[0:2, :], AF.Tanh, bias=CBs[0:2, 0:1])
                    tanh_instrs.append(ti)

                    # pool (banded matmuls), copy out, DMA
                    for ci, (T, od) in enumerate(((T_l[0], out0), (T_l[1], out1))):
                        po = gp.tile([P, 2 * D], f32, tag="G")
                        nc.tensor.matmul(po[:, 0:D], PTs[:, 0:P], T[:, 0:D],
                                         start=True, stop=False)
                        nc.tensor.matmul(po[:, 0:D], PTs[0:2, S:S + P], T[0:2, D:2 * D],
                                         start=False, stop=True)
                        nc.tensor.matmul(po[:, D:2 * D], PTs[:, S + P:2 * S], T[:, D:2 * D],
                                         start=True, stop=False)
                        nc.tensor.matmul(po[:, D:2 * D], PTt[:, P:S], Tt[0:2, ci * D:(ci + 1) * D],
                                         start=False, stop=True)
                        o = osp.tile([P, 2 * D], f32, tag="os")
                        if ci == 0:
                            nc.vector.tensor_copy(o[:], po[:])
                        else:
                            nc.scalar.copy(o[:], po[:])
                        nc.sync.dma_start(od[b].rearrange("(t p) d -> p t d", p=P), o[:].rearrange("p (t d) -> p t d", t=2))

        # force ACT ordering: all sqrt (sqrt table set) before all tanh (exp set)
        for ti in tanh_instrs:
            for si in sqrt_instrs:
                add_dep_helper(ti.ins, si.ins, sync=False, reason="act-table-phase")

    nc.compile()
    return nc


def _host_constants(W0, W1, conv_w, conv_b):
    k0 = np.asarray(conv_w)[0, 0, :, 0].astype(np.float64)
    k1 = np.asarray(conv_w)[0, 1, :, 0].astype(np.float64)
    c0t = np.zeros((S, H), np.float64)
    c1t = np.zeros((S, H), np.float64)
    for h in range(H):
        for w in range(W):
            s = h + w - (W - 1)
            if 0 <= s < S:
                c0t[s, h] = k0[w]
                c1t[s, h] = k1[w]
    pt = np.zeros((H, S), np.float64)
    for s in range(S):
        for t in range(W):
            pt[s + t, s] = 1.0 / W
    return {
        "w0bf": np.asarray(W0).astype(BF16),
        "w1bf": np.asarray(W1).astype(BF16),
        "c0t": c0t.astype(BF16),
        "c1t": c1t.astype(BF16),
        "ptd": pt.astype(BF16),
        "idp1": np.eye(P).astype(BF16),
        "idf": np.eye(P, dtype=np.float32),
        "ones1": np.ones((1, 2 * P), BF16),
        "cb": np.full((P, 1), np.asarray(conv_b)[0], np.float32),
    }


def kernel(F0r, F1r, sent0_mask, sent1_mask, W0, W1, conv_w, conv_b):
    _ensure_path()
    from concourse.bass_utils import run_bass_kernel_spmd

    if "nc" not in _CACHE:
        _CACHE["nc"] = _build_graph()
    nc = _CACHE["nc"]

    consts = _host_constants(W0, W1, conv_w, conv_b)
    in_maps = []
    for c in range(NCORES):
        sl = slice(c * BPC, (c + 1) * BPC)
        m = {
            "F0r": np.ascontiguousarray(np.asarray(F0r)[sl], np.float32),
            "F1r": np.ascontiguousarray(np.asarray(F1r)[sl], np.float32),
            "m0": np.ascontiguousarray(np.asarray(sent0_mask)[sl], np.float32),
            "m1": np.ascontiguousarray(np.asarray(sent1_mask)[sl], np.float32),
        }
        m.update(consts)
        in_maps.append(m)

    res = run_bass_kernel_spmd(nc, in_maps, core_ids=list(range(NCORES)))
    out0 = np.concatenate([r["out0"] for r in res.results], axis=0).astype(np.float32)
    out1 = np.concatenate([r["out1"] for r in res.results], axis=0).astype(np.float32)
    return out0, out1


if __name__ == "__main__":
    rng = np.random.default_rng(0)
    inputs = {
        "F0r": rng.standard_normal((B, S, D), dtype=np.float32),
        "F1r": rng.standard_normal((B, S, D), dtype=np.float32),
        "sent0_mask": (rng.random((B, S)) < 0.9).astype(np.float32),
        "sent1_mask": (rng.random((B, S)) < 0.9).astype(np.float32),
        "W0": rng.standard_normal((S, D), dtype=np.float32),
        "W1": rng.standard_normal((S, D), dtype=np.float32),
        "conv_w": rng.uniform(-0.4, 0.4, (1, 2, W, 1)).astype(np.float32),
        "conv_b": rng.uniform(-0.4, 0.4, (1,)).astype(np.float32),
    }
    o0, o1 = kernel(**inputs)
    print(o0.shape, o1.shape)
